# revision 7
# baseline (speedup 1.0000x reference)
"""Trainium2 Bass kernel for AttributeAttentionModule.

y = attention over heads of QKV projections:
  Q = sa @ Wq.T + bq ; K = x @ Wk.T + bk ; V = x @ Wv.T + bv   (all [B, D])
  per-sample scores[h,g] = Q_h . K_g / 32 ; softmax over g ; out_h = sum_g w_hg V_g

Data-parallel over 8 NeuronCores (batch sharded). Matmuls run in float32r
(FP22) at 1 cycle/row. Weights are streamed once per group of 8 batch-tiles
(all 8 PSUM banks accumulate in parallel over the contraction dim). All HBM
operands are pre-tiled on the host so every DMA descriptor is a contiguous
12KB-per-partition block. Attention is software-pipelined into the matmul
stream via filler chunks emitted after each o-sweep's PSUM copies.
"""

import os
import sys

for _p in ("/opt/trn_rl_repo", "/root/.axon_site/_ro/trn_rl_repo"):
    if os.path.isdir(_p) and _p not in sys.path:
        sys.path.append(_p)

import numpy as np
from contextlib import ExitStack

B = 16384
D = 3072
H = 3
DH = D // H          # 1024
NCORES = 8
P = 128              # partition tile
NO = 512             # matmul moving free dim (one PSUM bank of fp32)
KGRP = 6             # k-tiles per weight DMA
KT = D // P          # 24 contraction tiles
NOT = D // NO        # 6 output-column tiles
NKG = KT // KGRP     # 4 weight DMAs per o-column
KHALF = KT // 2      # stationary tiles arrive in two halves

_CACHE = {}


def _build(bs=B // NCORES, gbt=8):
    """Build + compile the per-core program. bs = batch rows per core,
    gbt = batch tiles (of 128) per weight-streaming group."""
    import concourse.bass as bass
    import concourse.tile as tile
    from concourse import bacc, mybir

    f32 = mybir.dt.float32
    f32r = mybir.dt.float32r
    mult = mybir.AluOpType.mult
    add = mybir.AluOpType.add
    bypass = mybir.AluOpType.bypass
    Exp = mybir.ActivationFunctionType.Exp

    nbt = bs // P        # batch tiles per core
    ng = nbt // gbt      # weight-stream groups

    nc = bacc.Bacc(
        "TRN2", target_bir_lowering=False, debug=False, num_devices=NCORES
    )

    # pre-tiled inputs (see kernel() for host layouts)
    sa4 = nc.dram_tensor("sa4", [nbt, P, KT, P], f32r, kind="ExternalInput").ap()
    x4 = nc.dram_tensor("x4", [nbt, P, KT, P], f32r, kind="ExternalInput").ap()
    wT = {
        t: nc.dram_tensor(
            f"w{t}5", [NOT, NKG, P, KGRP, NO], f32r, kind="ExternalInput"
        ).ap()
        for t in "qkv"
    }
    biasd = {
        t: nc.dram_tensor(f"b{t}", [P, D], f32, kind="ExternalInput").ap()
        for t in "qkv"
    }
    outd = nc.dram_tensor("out", [bs, D], f32, kind="ExternalOutput").ap()

    with tile.TileContext(nc) as tc, ExitStack() as ctx:
        dram = ctx.enter_context(tc.tile_pool(name="dram", bufs=1, space="DRAM"))
        qkv_s = {t: dram.tile([bs, D], f32, tag=f"s{t}", name=f"s{t}") for t in "qkv"}

        apool = ctx.enter_context(tc.tile_pool(name="apool", bufs=1))
        wpool = ctx.enter_context(tc.tile_pool(name="wpool", bufs=2))
        bpool = ctx.enter_context(tc.tile_pool(name="bpool", bufs=1))
        ocpool = ctx.enter_context(tc.tile_pool(name="ocpool", bufs=3))
        pspool = ctx.enter_context(tc.tile_pool(name="psum", bufs=1, space="PSUM"))
        qkvp = ctx.enter_context(tc.tile_pool(name="qkvp", bufs=1))
        smallp = ctx.enter_context(tc.tile_pool(name="smallp", bufs=4))
        accp = ctx.enter_context(tc.tile_pool(name="accp", bufs=2))
        prodp = ctx.enter_context(tc.tile_pool(name="prodp", bufs=1))
        outp = ctx.enter_context(tc.tile_pool(name="outp", bufs=1))

        pending = []  # attention chunk closures, drained between o-sweeps

        def filler():
            if pending:
                pending.pop(0)()

        def load_act(src, g):
            """Two half-k tiles per batch tile so matmuls can start on the
            first half while the second streams in."""
            los, his = [], []
            for i in range(gbt):
                lo = apool.tile([P, KHALF, P], f32r, tag=f"a{i}l", name=f"a{i}l")
                nc.gpsimd.dma_start(lo[:], src[g * gbt + i, :, 0:KHALF, :])
                los.append(lo)
            for i in range(gbt):
                hi = apool.tile([P, KHALF, P], f32r, tag=f"a{i}h", name=f"a{i}h")
                nc.gpsimd.dma_start(hi[:], src[g * gbt + i, :, KHALF:KT, :])
                his.append(hi)
            return list(zip(los, his))

        def proj(items, wTd, bias_d, dst, first_o_fill=True):
            """items: list of (global_bt_index, (a_lo, a_hi))."""
            bias_t = bpool.tile([P, D], f32, tag="bias", name="bias")
            nc.sync.dma_start(bias_t[:], bias_d[:])
            for o in range(NOT):
                ps = {
                    bt: pspool.tile([P, NO], f32, tag=f"ps{j}", name=f"ps{j}")
                    for j, (bt, _) in enumerate(items)
                }
                for kg in range(NKG):
                    wt = wpool.tile([P, KGRP, NO], f32r, tag="w", name="w")
                    nc.sync.dma_start(wt[:], wTd[o, kg])
                    for j in range(KGRP):
                        k = kg * KGRP + j
                        for bt, (alo, ahi) in items:
                            a = alo if k < KHALF else ahi
                            nc.tensor.matmul(
                                ps[bt][:],
                                a[:, k % KHALF, :],
                                wt[:, j, :],
                                start=(k == 0),
                                stop=(k == KT - 1),
                            )
                for bt, _ in items:
                    oc = ocpool.tile([P, NO], f32, tag="oc", name="oc")
                    nc.vector.tensor_add(
                        oc[:], ps[bt][:], bias_t[:, o * NO : (o + 1) * NO]
                    )
                    nc.gpsimd.dma_start(
                        dst[bt * P : bt * P + P, o * NO : (o + 1) * NO], oc[:]
                    )
                if first_o_fill or o > 0:
                    filler()

        def attn_chunks(bt):
            """Two closures per batch tile: A = load + scores + softmax,
            B = weighted V combine + store."""
            r0 = bt * P
            t3 = {}
            small = {}

            def chunk_a():
                for t in "qkv":
                    tt = qkvp.tile([P, D], f32, tag=t, name=f"t_{t}")
                    nc.scalar.dma_start(tt[:], qkv_s[t][r0 : r0 + P, :])
                    t3[t] = tt
                s = smallp.tile([P, H * H], f32, tag="s", name="s")
                prod = prodp.tile([P, DH], f32, tag="prod", name="prod")
                for h in range(H):
                    for g2 in range(H):
                        # fused row-wise dot: prod = Q_h*K_g ; s_hg = sum(prod)
                        nc.vector.scalar_tensor_tensor(
                            prod[:],
                            t3["q"][:, h * DH : (h + 1) * DH],
                            1.0,
                            t3["k"][:, g2 * DH : (g2 + 1) * DH],
                            op0=bypass,
                            op1=mult,
                            accum_out=s[:, h * H + g2 : h * H + g2 + 1],
                        )
                e = smallp.tile([P, H * H], f32, tag="e", name="e")
                nc.scalar.activation(e[:], s[:], Exp, scale=1.0 / 32.0)
                ssum = smallp.tile([P, H], f32, tag="ssum", name="ssum")
                nc.vector.tensor_reduce(
                    ssum[:],
                    e[:].rearrange("p (h g) -> p h g", h=H),
                    axis=mybir.AxisListType.X,
                    op=add,
                )
                rcp = smallp.tile([P, H], f32, tag="rcp", name="rcp")
                nc.vector.reciprocal(rcp[:], ssum[:])
                small["e"] = e
                small["rcp"] = rcp

            def chunk_b():
                e, rcp = small["e"], small["rcp"]
                ot = outp.tile([P, D], f32, tag="o", name="o")
                for h in range(H):
                    acc = accp.tile([P, DH], f32, tag="acc", name="acc")
                    # first term on ScalarE (per-partition scalar scale)
                    nc.scalar.mul(acc[:], t3["v"][:, 0:DH], e[:, h * H : h * H + 1])
                    for g2 in (1, 2):
                        nc.vector.scalar_tensor_tensor(
                            acc[:],
                            t3["v"][:, g2 * DH : (g2 + 1) * DH],
                            e[:, h * H + g2 : h * H + g2 + 1],
                            acc[:],
                            op0=mult,
                            op1=add,
                        )
                    nc.scalar.mul(
                        ot[:, h * DH : (h + 1) * DH], acc[:], rcp[:, h : h + 1]
                    )
                nc.scalar.dma_start(outd[r0 : r0 + P, :], ot[:])

            return [chunk_a, chunk_b]

        for g in range(ng):
            last = g == ng - 1
            bts = [g * gbt + i for i in range(gbt)]
            sa_t = load_act(sa4, g)
            proj(list(zip(bts, sa_t)), wT["q"], biasd["q"], qkv_s["q"])
            x_t = load_act(x4, g)
            proj(list(zip(bts, x_t)), wT["k"], biasd["k"], qkv_s["k"])
            items = list(zip(bts, x_t))
            if last and gbt >= 2:
                half = gbt // 2
                proj(items[:half], wT["v"], biasd["v"], qkv_s["v"])
                for bt in bts[:half]:
                    pending.extend(attn_chunks(bt))
                proj(items[half:], wT["v"], biasd["v"], qkv_s["v"])
                for bt in bts[half:]:
                    pending.extend(attn_chunks(bt))
            else:
                proj(items, wT["v"], biasd["v"], qkv_s["v"])
                for bt in bts:
                    pending.extend(attn_chunks(bt))
        while pending:
            pending.pop(0)()

    nc.compile()
    return nc


def _get_nc(bs=B // NCORES, gbt=8):
    key = (bs, gbt)
    if key not in _CACHE:
        _CACHE[key] = _build(bs, gbt)
    return _CACHE[key]


def _prep_weights(Wq, Wk, Wv, bq, bk, bv):
    """Pre-tile weights: w5[o, kg, p, j, n] = W.T[(kg*KGRP+j)*P + p, o*NO + n]."""
    ws = {}
    for nm, W in (("q", Wq), ("k", Wk), ("v", Wv)):
        wt = np.asarray(W, dtype=np.float32).T  # [in, out]
        w5 = wt.reshape(NKG, KGRP, P, NOT, NO).transpose(3, 0, 2, 1, 4)
        ws[nm] = np.ascontiguousarray(w5)
    bb = {
        nm: np.ascontiguousarray(
            np.broadcast_to(np.asarray(b, dtype=np.float32), (P, D))
        )
        for nm, b in (("q", bq), ("k", bk), ("v", bv))
    }
    return ws, bb


def _prep_act(a, bs):
    """Pre-tile activations per core: a4[bt, p, ko, b] = a[bt*P + b, ko*P + p]."""
    nbt = bs // P
    a4 = a.reshape(nbt, P, KT, P).transpose(0, 3, 2, 1)
    return np.ascontiguousarray(a4)


def _in_maps(x, sa, ws, bb, bs):
    maps = []
    for c in range(NCORES):
        r0 = c * bs
        maps.append(
            {
                "sa4": _prep_act(sa[r0 : r0 + bs], bs),
                "x4": _prep_act(x[r0 : r0 + bs], bs),
                "wq5": ws["q"],
                "wk5": ws["k"],
                "wv5": ws["v"],
                "bq": bb["q"],
                "bk": bb["k"],
                "bv": bb["v"],
            }
        )
    return maps


def kernel(x, synthetic_attributes, Wq, bq, Wk, bk, Wv, bv, **_ignored):
    from concourse import bass_utils

    x = np.asarray(x, dtype=np.float32)
    sa = np.asarray(synthetic_attributes, dtype=np.float32)
    bs = x.shape[0] // NCORES

    ws, bb = _prep_weights(Wq, Wk, Wv, bq, bk, bv)
    nc = _get_nc(bs=bs)
    in_maps = _in_maps(x, sa, ws, bb, bs)

    res = bass_utils.run_bass_kernel_spmd(nc, in_maps, core_ids=list(range(NCORES)))
    out = np.concatenate([res.results[c]["out"] for c in range(NCORES)], axis=0)
    return out


# revision 9
# speedup vs baseline: 1.0190x; 1.0190x over previous
"""Trainium2 Bass kernel for AttributeAttentionModule.

y = attention over heads of QKV projections:
  Q = sa @ Wq.T + bq ; K = x @ Wk.T + bk ; V = x @ Wv.T + bv   (all [B, D])
  per-sample scores[h,g] = Q_h . K_g / 32 ; softmax over g ; out_h = sum_g w_hg V_g

Data-parallel over 8 NeuronCores (batch sharded). Matmuls run in float32r
(FP22) at 1 cycle/row. Weights are streamed once per group of 8 batch-tiles
(all 8 PSUM banks accumulate in parallel over the contraction dim). All HBM
operands are pre-tiled on the host so every DMA descriptor is a contiguous
12KB-per-partition block. Attention is software-pipelined into the matmul
stream via filler chunks emitted after each o-sweep's PSUM copies.
"""

import os
import sys

for _p in ("/opt/trn_rl_repo", "/root/.axon_site/_ro/trn_rl_repo"):
    if os.path.isdir(_p) and _p not in sys.path:
        sys.path.append(_p)

import numpy as np
from contextlib import ExitStack

B = 16384
D = 3072
H = 3
DH = D // H          # 1024
NCORES = 8
P = 128              # partition tile
NO = 512             # matmul moving free dim (one PSUM bank of fp32)
KGRP = 6             # k-tiles per weight DMA
KT = D // P          # 24 contraction tiles
NOT = D // NO        # 6 output-column tiles
NKG = KT // KGRP     # 4 weight DMAs per o-column
KHALF = KT // 2      # stationary tiles arrive in two halves

_CACHE = {}


def _build(bs=B // NCORES, gbt=8):
    """Build + compile the per-core program. bs = batch rows per core,
    gbt = batch tiles (of 128) per weight-streaming group."""
    import concourse.bass as bass
    import concourse.tile as tile
    from concourse import bacc, mybir

    f32 = mybir.dt.float32
    f32r = mybir.dt.float32r
    mult = mybir.AluOpType.mult
    add = mybir.AluOpType.add
    bypass = mybir.AluOpType.bypass
    Exp = mybir.ActivationFunctionType.Exp

    nbt = bs // P        # batch tiles per core
    ng = nbt // gbt      # weight-stream groups

    nc = bacc.Bacc(
        "TRN2", target_bir_lowering=False, debug=False, num_devices=NCORES
    )

    # pre-tiled inputs (see kernel() for host layouts)
    sa4 = nc.dram_tensor("sa4", [nbt, P, KT, P], f32r, kind="ExternalInput").ap()
    x4 = nc.dram_tensor("x4", [nbt, P, KT, P], f32r, kind="ExternalInput").ap()
    wT = {
        t: nc.dram_tensor(
            f"w{t}5", [NOT, NKG, P, KGRP, NO], f32r, kind="ExternalInput"
        ).ap()
        for t in "qkv"
    }
    biasd = {
        t: nc.dram_tensor(f"b{t}", [P, D], f32, kind="ExternalInput").ap()
        for t in "qkv"
    }
    outd = nc.dram_tensor("out", [bs, D], f32, kind="ExternalOutput").ap()

    with tile.TileContext(nc) as tc, ExitStack() as ctx:
        dram = ctx.enter_context(tc.tile_pool(name="dram", bufs=1, space="DRAM"))
        qkv_s = {t: dram.tile([bs, D], f32, tag=f"s{t}", name=f"s{t}") for t in "qkv"}

        apool = ctx.enter_context(tc.tile_pool(name="apool", bufs=1))
        wpool = ctx.enter_context(tc.tile_pool(name="wpool", bufs=2))
        bpool = ctx.enter_context(tc.tile_pool(name="bpool", bufs=1))
        ocpool = ctx.enter_context(tc.tile_pool(name="ocpool", bufs=3))
        pspool = ctx.enter_context(tc.tile_pool(name="psum", bufs=1, space="PSUM"))
        qkvp = ctx.enter_context(tc.tile_pool(name="qkvp", bufs=1))
        smallp = ctx.enter_context(tc.tile_pool(name="smallp", bufs=4))
        accp = ctx.enter_context(tc.tile_pool(name="accp", bufs=2))
        prodp = ctx.enter_context(tc.tile_pool(name="prodp", bufs=1))
        outp = ctx.enter_context(tc.tile_pool(name="outp", bufs=1))

        pending = []  # attention chunk closures, drained between o-sweeps

        def filler():
            if pending:
                pending.pop(0)()

        def load_act(src, g):
            """Two half-k tiles per batch tile so matmuls can start on the
            first half while the second streams in."""
            los, his = [], []
            for i in range(gbt):
                lo = apool.tile([P, KHALF, P], f32r, tag=f"a{i}l", name=f"a{i}l")
                nc.gpsimd.dma_start(lo[:], src[g * gbt + i, :, 0:KHALF, :])
                los.append(lo)
            for i in range(gbt):
                hi = apool.tile([P, KHALF, P], f32r, tag=f"a{i}h", name=f"a{i}h")
                nc.gpsimd.dma_start(hi[:], src[g * gbt + i, :, KHALF:KT, :])
                his.append(hi)
            return list(zip(los, his))

        def proj(items, wTd, bias_d, dst, first_o_fill=True):
            """items: list of (global_bt_index, (a_lo, a_hi))."""
            bias_t = bpool.tile([P, D], f32, tag="bias", name="bias")
            nc.sync.dma_start(bias_t[:], bias_d[:])
            for o in range(NOT):
                ps = {
                    bt: pspool.tile([P, NO], f32, tag=f"ps{j}", name=f"ps{j}")
                    for j, (bt, _) in enumerate(items)
                }
                for kg in range(NKG):
                    wt = wpool.tile([P, KGRP, NO], f32r, tag="w", name="w")
                    nc.sync.dma_start(wt[:], wTd[o, kg])
                    for j in range(KGRP):
                        k = kg * KGRP + j
                        for bt, (alo, ahi) in items:
                            a = alo if k < KHALF else ahi
                            nc.tensor.matmul(
                                ps[bt][:],
                                a[:, k % KHALF, :],
                                wt[:, j, :],
                                start=(k == 0),
                                stop=(k == KT - 1),
                            )
                for bt, _ in items:
                    oc = ocpool.tile([P, NO], f32, tag="oc", name="oc")
                    nc.vector.tensor_add(
                        oc[:], ps[bt][:], bias_t[:, o * NO : (o + 1) * NO]
                    )
                    nc.scalar.dma_start(
                        dst[bt * P : bt * P + P, o * NO : (o + 1) * NO], oc[:]
                    )
                if first_o_fill or o > 0:
                    filler()

        def attn_chunks(bt):
            """Two closures per batch tile: A = load + scores + softmax,
            B = weighted V combine + store."""
            r0 = bt * P
            t3 = {}
            small = {}

            def chunk_a():
                for t in "qkv":
                    tt = qkvp.tile([P, D], f32, tag=t, name=f"t_{t}")
                    nc.scalar.dma_start(tt[:], qkv_s[t][r0 : r0 + P, :])
                    t3[t] = tt
                s = smallp.tile([P, H * H], f32, tag="s", name="s")
                prod = prodp.tile([P, DH], f32, tag="prod", name="prod")
                for h in range(H):
                    for g2 in range(H):
                        # fused row-wise dot: prod = Q_h*K_g ; s_hg = sum(prod)
                        nc.vector.scalar_tensor_tensor(
                            prod[:],
                            t3["q"][:, h * DH : (h + 1) * DH],
                            1.0,
                            t3["k"][:, g2 * DH : (g2 + 1) * DH],
                            op0=bypass,
                            op1=mult,
                            accum_out=s[:, h * H + g2 : h * H + g2 + 1],
                        )
                e = smallp.tile([P, H * H], f32, tag="e", name="e")
                nc.scalar.activation(e[:], s[:], Exp, scale=1.0 / 32.0)
                ssum = smallp.tile([P, H], f32, tag="ssum", name="ssum")
                nc.vector.tensor_reduce(
                    ssum[:],
                    e[:].rearrange("p (h g) -> p h g", h=H),
                    axis=mybir.AxisListType.X,
                    op=add,
                )
                rcp = smallp.tile([P, H], f32, tag="rcp", name="rcp")
                nc.vector.reciprocal(rcp[:], ssum[:])
                small["e"] = e
                small["rcp"] = rcp

            def chunk_b():
                e, rcp = small["e"], small["rcp"]
                ot = outp.tile([P, D], f32, tag="o", name="o")
                for h in range(H):
                    acc = accp.tile([P, DH], f32, tag="acc", name="acc")
                    # first term on ScalarE (per-partition scalar scale)
                    nc.scalar.mul(acc[:], t3["v"][:, 0:DH], e[:, h * H : h * H + 1])
                    for g2 in (1, 2):
                        nc.vector.scalar_tensor_tensor(
                            acc[:],
                            t3["v"][:, g2 * DH : (g2 + 1) * DH],
                            e[:, h * H + g2 : h * H + g2 + 1],
                            acc[:],
                            op0=mult,
                            op1=add,
                        )
                    nc.scalar.mul(
                        ot[:, h * DH : (h + 1) * DH], acc[:], rcp[:, h : h + 1]
                    )
                nc.scalar.dma_start(outd[r0 : r0 + P, :], ot[:])

            return [chunk_a, chunk_b]

        for g in range(ng):
            last = g == ng - 1
            bts = [g * gbt + i for i in range(gbt)]
            sa_t = load_act(sa4, g)
            proj(list(zip(bts, sa_t)), wT["q"], biasd["q"], qkv_s["q"])
            x_t = load_act(x4, g)
            proj(list(zip(bts, x_t)), wT["k"], biasd["k"], qkv_s["k"])
            items = list(zip(bts, x_t))
            if last and gbt >= 2:
                half = gbt // 2
                proj(items[:half], wT["v"], biasd["v"], qkv_s["v"])
                for bt in bts[:half]:
                    pending.extend(attn_chunks(bt))
                proj(items[half:], wT["v"], biasd["v"], qkv_s["v"])
                for bt in bts[half:]:
                    pending.extend(attn_chunks(bt))
            else:
                proj(items, wT["v"], biasd["v"], qkv_s["v"])
                for bt in bts:
                    pending.extend(attn_chunks(bt))
        while pending:
            pending.pop(0)()

    nc.compile()
    return nc


def _get_nc(bs=B // NCORES, gbt=8):
    key = (bs, gbt)
    if key not in _CACHE:
        _CACHE[key] = _build(bs, gbt)
    return _CACHE[key]


def _prep_weights(Wq, Wk, Wv, bq, bk, bv):
    """Pre-tile weights: w5[o, kg, p, j, n] = W.T[(kg*KGRP+j)*P + p, o*NO + n]."""
    ws = {}
    for nm, W in (("q", Wq), ("k", Wk), ("v", Wv)):
        wt = np.asarray(W, dtype=np.float32).T  # [in, out]
        w5 = wt.reshape(NKG, KGRP, P, NOT, NO).transpose(3, 0, 2, 1, 4)
        ws[nm] = np.ascontiguousarray(w5)
    bb = {
        nm: np.ascontiguousarray(
            np.broadcast_to(np.asarray(b, dtype=np.float32), (P, D))
        )
        for nm, b in (("q", bq), ("k", bk), ("v", bv))
    }
    return ws, bb


def _prep_act(a, bs):
    """Pre-tile activations per core: a4[bt, p, ko, b] = a[bt*P + b, ko*P + p]."""
    nbt = bs // P
    a4 = a.reshape(nbt, P, KT, P).transpose(0, 3, 2, 1)
    return np.ascontiguousarray(a4)


def _in_maps(x, sa, ws, bb, bs):
    maps = []
    for c in range(NCORES):
        r0 = c * bs
        maps.append(
            {
                "sa4": _prep_act(sa[r0 : r0 + bs], bs),
                "x4": _prep_act(x[r0 : r0 + bs], bs),
                "wq5": ws["q"],
                "wk5": ws["k"],
                "wv5": ws["v"],
                "bq": bb["q"],
                "bk": bb["k"],
                "bv": bb["v"],
            }
        )
    return maps


def kernel(x, synthetic_attributes, Wq, bq, Wk, bk, Wv, bv, **_ignored):
    from concourse import bass_utils

    x = np.asarray(x, dtype=np.float32)
    sa = np.asarray(synthetic_attributes, dtype=np.float32)
    bs = x.shape[0] // NCORES

    ws, bb = _prep_weights(Wq, Wk, Wv, bq, bk, bv)
    nc = _get_nc(bs=bs)
    in_maps = _in_maps(x, sa, ws, bb, bs)

    res = bass_utils.run_bass_kernel_spmd(nc, in_maps, core_ids=list(range(NCORES)))
    out = np.concatenate([res.results[c]["out"] for c in range(NCORES)], axis=0)
    return out


# revision 10
# speedup vs baseline: 1.0569x; 1.0372x over previous
"""Trainium2 Bass kernel for AttributeAttentionModule.

y = attention over heads of QKV projections:
  Q = sa @ Wq.T + bq ; K = x @ Wk.T + bk ; V = x @ Wv.T + bv   (all [B, D])
  per-sample scores[h,g] = Q_h . K_g / 32 ; softmax over g ; out_h = sum_g w_hg V_g

Data-parallel over 8 NeuronCores (batch sharded). Matmuls run in float32r
(FP22) at 1 cycle/row. Weights are streamed once per group of 8 batch-tiles
(all 8 PSUM banks accumulate in parallel over the contraction dim). All HBM
operands are pre-tiled on the host so every DMA descriptor is a contiguous
12KB-per-partition block. Attention is software-pipelined into the matmul
stream via filler chunks emitted after each o-sweep's PSUM copies.
"""

import os
import sys

for _p in ("/opt/trn_rl_repo", "/root/.axon_site/_ro/trn_rl_repo"):
    if os.path.isdir(_p) and _p not in sys.path:
        sys.path.append(_p)

import numpy as np
from contextlib import ExitStack

B = 16384
D = 3072
H = 3
DH = D // H          # 1024
NCORES = 8
P = 128              # partition tile
NO = 512             # matmul moving free dim (one PSUM bank of fp32)
KGRP = 3             # k-tiles per weight DMA
KT = D // P          # 24 contraction tiles
NOT = D // NO        # 6 output-column tiles
NKG = KT // KGRP     # 4 weight DMAs per o-column
KHALF = KT // 2      # stationary tiles arrive in two halves

_CACHE = {}


def _build(bs=B // NCORES, gbt=8):
    """Build + compile the per-core program. bs = batch rows per core,
    gbt = batch tiles (of 128) per weight-streaming group."""
    import concourse.bass as bass
    import concourse.tile as tile
    from concourse import bacc, mybir

    f32 = mybir.dt.float32
    f32r = mybir.dt.float32r
    mult = mybir.AluOpType.mult
    add = mybir.AluOpType.add
    bypass = mybir.AluOpType.bypass
    Exp = mybir.ActivationFunctionType.Exp

    nbt = bs // P        # batch tiles per core
    ng = nbt // gbt      # weight-stream groups

    nc = bacc.Bacc(
        "TRN2", target_bir_lowering=False, debug=False, num_devices=NCORES
    )

    # pre-tiled inputs (see kernel() for host layouts)
    sa4 = nc.dram_tensor("sa4", [nbt, P, KT, P], f32r, kind="ExternalInput").ap()
    x4 = nc.dram_tensor("x4", [nbt, P, KT, P], f32r, kind="ExternalInput").ap()
    wT = {
        t: nc.dram_tensor(
            f"w{t}5", [NOT, NKG, P, KGRP, NO], f32r, kind="ExternalInput"
        ).ap()
        for t in "qkv"
    }
    biasd = {
        t: nc.dram_tensor(f"b{t}", [P, D], f32, kind="ExternalInput").ap()
        for t in "qkv"
    }
    outd = nc.dram_tensor("out", [bs, D], f32, kind="ExternalOutput").ap()

    with tile.TileContext(nc) as tc, ExitStack() as ctx:
        dram = ctx.enter_context(tc.tile_pool(name="dram", bufs=1, space="DRAM"))
        qkv_s = {t: dram.tile([bs, D], f32, tag=f"s{t}", name=f"s{t}") for t in "qkv"}

        apool = ctx.enter_context(tc.tile_pool(name="apool", bufs=1))
        wpool = ctx.enter_context(tc.tile_pool(name="wpool", bufs=4))
        bpool = ctx.enter_context(tc.tile_pool(name="bpool", bufs=1))
        ocpool = ctx.enter_context(tc.tile_pool(name="ocpool", bufs=3))
        pspool = ctx.enter_context(tc.tile_pool(name="psum", bufs=1, space="PSUM"))
        qkvp = ctx.enter_context(tc.tile_pool(name="qkvp", bufs=1))
        smallp = ctx.enter_context(tc.tile_pool(name="smallp", bufs=4))
        accp = ctx.enter_context(tc.tile_pool(name="accp", bufs=2))
        prodp = ctx.enter_context(tc.tile_pool(name="prodp", bufs=1))
        outp = ctx.enter_context(tc.tile_pool(name="outp", bufs=1))

        pending = []  # attention chunk closures, drained between o-sweeps

        def filler():
            if pending:
                pending.pop(0)()

        def load_act(src, g):
            """Two half-k tiles per batch tile so matmuls can start on the
            first half while the second streams in."""
            los, his = [], []
            for i in range(gbt):
                lo = apool.tile([P, KHALF, P], f32r, tag=f"a{i}l", name=f"a{i}l")
                nc.gpsimd.dma_start(lo[:], src[g * gbt + i, :, 0:KHALF, :])
                los.append(lo)
            for i in range(gbt):
                hi = apool.tile([P, KHALF, P], f32r, tag=f"a{i}h", name=f"a{i}h")
                nc.gpsimd.dma_start(hi[:], src[g * gbt + i, :, KHALF:KT, :])
                his.append(hi)
            return list(zip(los, his))

        def proj(items, wTd, bias_d, dst, first_o_fill=True):
            """items: list of (global_bt_index, (a_lo, a_hi))."""
            bias_t = bpool.tile([P, D], f32, tag="bias", name="bias")
            nc.sync.dma_start(bias_t[:], bias_d[:])
            for o in range(NOT):
                ps = {
                    bt: pspool.tile([P, NO], f32, tag=f"ps{j}", name=f"ps{j}")
                    for j, (bt, _) in enumerate(items)
                }
                for kg in range(NKG):
                    wt = wpool.tile([P, KGRP, NO], f32r, tag="w", name="w")
                    nc.sync.dma_start(wt[:], wTd[o, kg])
                    for j in range(KGRP):
                        k = kg * KGRP + j
                        for bt, (alo, ahi) in items:
                            a = alo if k < KHALF else ahi
                            nc.tensor.matmul(
                                ps[bt][:],
                                a[:, k % KHALF, :],
                                wt[:, j, :],
                                start=(k == 0),
                                stop=(k == KT - 1),
                            )
                for bt, _ in items:
                    oc = ocpool.tile([P, NO], f32, tag="oc", name="oc")
                    nc.vector.tensor_add(
                        oc[:], ps[bt][:], bias_t[:, o * NO : (o + 1) * NO]
                    )
                    nc.scalar.dma_start(
                        dst[bt * P : bt * P + P, o * NO : (o + 1) * NO], oc[:]
                    )
                if first_o_fill or o > 0:
                    filler()

        def attn_chunks(bt):
            """Two closures per batch tile: A = load + scores + softmax,
            B = weighted V combine + store."""
            r0 = bt * P
            t3 = {}
            small = {}

            def chunk_a():
                for t in "qkv":
                    tt = qkvp.tile([P, D], f32, tag=t, name=f"t_{t}")
                    nc.scalar.dma_start(tt[:], qkv_s[t][r0 : r0 + P, :])
                    t3[t] = tt
                s = smallp.tile([P, H * H], f32, tag="s", name="s")
                prod = prodp.tile([P, DH], f32, tag="prod", name="prod")
                for h in range(H):
                    for g2 in range(H):
                        # fused row-wise dot: prod = Q_h*K_g ; s_hg = sum(prod)
                        nc.vector.scalar_tensor_tensor(
                            prod[:],
                            t3["q"][:, h * DH : (h + 1) * DH],
                            1.0,
                            t3["k"][:, g2 * DH : (g2 + 1) * DH],
                            op0=bypass,
                            op1=mult,
                            accum_out=s[:, h * H + g2 : h * H + g2 + 1],
                        )
                e = smallp.tile([P, H * H], f32, tag="e", name="e")
                nc.scalar.activation(e[:], s[:], Exp, scale=1.0 / 32.0)
                ssum = smallp.tile([P, H], f32, tag="ssum", name="ssum")
                nc.vector.tensor_reduce(
                    ssum[:],
                    e[:].rearrange("p (h g) -> p h g", h=H),
                    axis=mybir.AxisListType.X,
                    op=add,
                )
                rcp = smallp.tile([P, H], f32, tag="rcp", name="rcp")
                nc.vector.reciprocal(rcp[:], ssum[:])
                small["e"] = e
                small["rcp"] = rcp

            def chunk_b():
                e, rcp = small["e"], small["rcp"]
                ot = outp.tile([P, D], f32, tag="o", name="o")
                for h in range(H):
                    acc = accp.tile([P, DH], f32, tag="acc", name="acc")
                    # first term on ScalarE (per-partition scalar scale)
                    nc.scalar.mul(acc[:], t3["v"][:, 0:DH], e[:, h * H : h * H + 1])
                    for g2 in (1, 2):
                        nc.vector.scalar_tensor_tensor(
                            acc[:],
                            t3["v"][:, g2 * DH : (g2 + 1) * DH],
                            e[:, h * H + g2 : h * H + g2 + 1],
                            acc[:],
                            op0=mult,
                            op1=add,
                        )
                    nc.scalar.mul(
                        ot[:, h * DH : (h + 1) * DH], acc[:], rcp[:, h : h + 1]
                    )
                nc.scalar.dma_start(outd[r0 : r0 + P, :], ot[:])

            return [chunk_a, chunk_b]

        for g in range(ng):
            last = g == ng - 1
            bts = [g * gbt + i for i in range(gbt)]
            sa_t = load_act(sa4, g)
            proj(list(zip(bts, sa_t)), wT["q"], biasd["q"], qkv_s["q"])
            x_t = load_act(x4, g)
            proj(list(zip(bts, x_t)), wT["k"], biasd["k"], qkv_s["k"])
            items = list(zip(bts, x_t))
            if last and gbt >= 2:
                half = gbt // 2
                proj(items[:half], wT["v"], biasd["v"], qkv_s["v"])
                for bt in bts[:half]:
                    pending.extend(attn_chunks(bt))
                proj(items[half:], wT["v"], biasd["v"], qkv_s["v"])
                for bt in bts[half:]:
                    pending.extend(attn_chunks(bt))
            else:
                proj(items, wT["v"], biasd["v"], qkv_s["v"])
                for bt in bts:
                    pending.extend(attn_chunks(bt))
        while pending:
            pending.pop(0)()

    nc.compile()
    return nc


def _get_nc(bs=B // NCORES, gbt=8):
    key = (bs, gbt)
    if key not in _CACHE:
        _CACHE[key] = _build(bs, gbt)
    return _CACHE[key]


def _prep_weights(Wq, Wk, Wv, bq, bk, bv):
    """Pre-tile weights: w5[o, kg, p, j, n] = W.T[(kg*KGRP+j)*P + p, o*NO + n]."""
    ws = {}
    for nm, W in (("q", Wq), ("k", Wk), ("v", Wv)):
        wt = np.asarray(W, dtype=np.float32).T  # [in, out]
        w5 = wt.reshape(NKG, KGRP, P, NOT, NO).transpose(3, 0, 2, 1, 4)
        ws[nm] = np.ascontiguousarray(w5)
    bb = {
        nm: np.ascontiguousarray(
            np.broadcast_to(np.asarray(b, dtype=np.float32), (P, D))
        )
        for nm, b in (("q", bq), ("k", bk), ("v", bv))
    }
    return ws, bb


def _prep_act(a, bs):
    """Pre-tile activations per core: a4[bt, p, ko, b] = a[bt*P + b, ko*P + p]."""
    nbt = bs // P
    a4 = a.reshape(nbt, P, KT, P).transpose(0, 3, 2, 1)
    return np.ascontiguousarray(a4)


def _in_maps(x, sa, ws, bb, bs):
    maps = []
    for c in range(NCORES):
        r0 = c * bs
        maps.append(
            {
                "sa4": _prep_act(sa[r0 : r0 + bs], bs),
                "x4": _prep_act(x[r0 : r0 + bs], bs),
                "wq5": ws["q"],
                "wk5": ws["k"],
                "wv5": ws["v"],
                "bq": bb["q"],
                "bk": bb["k"],
                "bv": bb["v"],
            }
        )
    return maps


def kernel(x, synthetic_attributes, Wq, bq, Wk, bk, Wv, bv, **_ignored):
    from concourse import bass_utils

    x = np.asarray(x, dtype=np.float32)
    sa = np.asarray(synthetic_attributes, dtype=np.float32)
    bs = x.shape[0] // NCORES

    ws, bb = _prep_weights(Wq, Wk, Wv, bq, bk, bv)
    nc = _get_nc(bs=bs)
    in_maps = _in_maps(x, sa, ws, bb, bs)

    res = bass_utils.run_bass_kernel_spmd(nc, in_maps, core_ids=list(range(NCORES)))
    out = np.concatenate([res.results[c]["out"] for c in range(NCORES)], axis=0)
    return out
